# revision 39
# baseline (speedup 1.0000x reference)
"""Trainium2 Bass kernel for nn_Attention (cumulative masked softmax attention).

Reference computation:
    v   = tanh(x @ W + b)                  (B, T, F)
    a   = v . u                            (B, T)   -- query-independent logits
    e   = exp(a)[:, None, :] * tril * mask (B, T, T)
    alf = e / (sum_s e + EPS)
    c   = alf @ x                          (B, T, F)

Because the logits are query-independent and the mask is lower-triangular,
the (B,T,T) softmax-matmul collapses to a running weighted average:
    w[s]  = exp(a[s]) * mask[s]
    c[t]  = cumsum_s(w * x)[t] / (cumsum_s(w)[t] + EPS)
which is O(B*T*F) instead of O(B*T^2*F).

Sharding: data-parallel over batch B across 8 NeuronCores (2 batches/core).
W/u/b replicated. Each core processes 2048 rows of (T, F); the (w*x) cumsum
is done per-batch with triangular/ones matmul blocks on the tensor engine,
and the scalar cumsum of w via two tiny matmuls + a free-dim prefix scan.
Matmul operands use float32r (fp32, ~11-bit mantissa, full PE rate).
The host supplies x both in natural layout and pre-transposed (xT) so the
tensor engine needs no on-chip transposes for the x @ W contraction.
"""

import numpy as np

import concourse.bass as bass  # noqa: F401
import concourse.tile as tile
from concourse import bacc, mybir
from concourse.bass_utils import run_bass_kernel_spmd

B, T, F = 16, 1024, 512
EPS = 1e-7
NCORES = 8
B_LOC = B // NCORES          # batches per core
R = B_LOC * T                # rows per core
P = 128                      # partition tile
NT = R // P                  # row tiles per core
NTB = T // P                 # row tiles per batch
KC = F // P                  # contraction chunks

F32 = mybir.dt.float32
F32R = mybir.dt.float32r


def _build(have_b: bool, have_mask: bool, loop_n: int = 0):
    """Build the per-core Bass module. loop_n > 0 wraps the body in a
    hardware For_i loop (used only for timing)."""
    nc = bacc.Bacc("TRN2", target_bir_lowering=False, debug=False)

    x_d = nc.dram_tensor("x", [NT, P, F], F32, kind="ExternalInput")
    xt_d = nc.dram_tensor("xT", [NT, P, F], F32R, kind="ExternalInput")
    # W pre-arranged on host as (P, KC*F): W_host[p, k*F+f] = W[k*P+p, f]
    w_d = nc.dram_tensor("W", [P, KC * F], F32R, kind="ExternalInput")
    u_d = nc.dram_tensor("u", [1, F], F32, kind="ExternalInput")
    if have_b:
        b_d = nc.dram_tensor("b", [1, F], F32, kind="ExternalInput")
    if have_mask:
        m_d = nc.dram_tensor("m", [NT, P, 1], F32, kind="ExternalInput")
    c_d = nc.dram_tensor("c", [NT, P, F], F32, kind="ExternalOutput")

    Tanh = mybir.ActivationFunctionType.Tanh
    Exp = mybir.ActivationFunctionType.Exp
    Copy = mybir.ActivationFunctionType.Copy
    ADD = mybir.AluOpType.add
    SUB = mybir.AluOpType.subtract

    NP = NT // 2             # tile pairs

    with tile.TileContext(nc) as tc:
        with (
            tc.tile_pool(name="const", bufs=1) as const,
            tc.tile_pool(name="xp", bufs=6) as xp,
            tc.tile_pool(name="xtp", bufs=6) as xtp,
            tc.tile_pool(name="vp", bufs=2) as vp,
            tc.tile_pool(name="scrp", bufs=2) as scrp,
            tc.tile_pool(name="yp", bufs=NT) as yp,
            tc.tile_pool(name="wap", bufs=B_LOC) as wap,
            tc.tile_pool(name="smal", bufs=6) as smal,
            tc.tile_pool(name="cp", bufs=3) as cp,
            tc.tile_pool(name="ps_v", bufs=2, space="PSUM") as ps_v_pool,
            tc.tile_pool(name="ps_P", bufs=2, space="PSUM") as ps_P_pool,
            tc.tile_pool(name="ps_Z", bufs=1, space="PSUM") as ps_Z_pool,
        ):
            # ---- constants ----
            W_sb = const.tile([P, KC, F], F32R)
            nc.sync.dma_start(out=W_sb,
                              in_=w_d.ap().rearrange("p (k f) -> p k f", k=KC))
            u_bc2 = const.tile([P, 2, F], F32)
            nc.gpsimd.dma_start(out=u_bc2[:, 0, :],
                                in_=u_d.ap().to_broadcast((P, F)))
            nc.gpsimd.dma_start(out=u_bc2[:, 1, :],
                                in_=u_d.ap().to_broadcast((P, F)))
            if have_b:
                b_sb = const.tile([1, F], F32R)
                bf = smal.tile([1, F], F32, tag="bstage")
                nc.sync.dma_start(out=bf, in_=b_d.ap())
                nc.vector.tensor_copy(b_sb, bf)
                ones_row = const.tile([1, P], F32R)
                nc.vector.memset(ones_row, 1.0)
            # triangular + ones matmul weights for the cumsum (exact in f32r)
            triu_f = const.tile([P, P], F32)
            nc.gpsimd.memset(triu_f, 0.0)
            nc.gpsimd.affine_select(
                out=triu_f, in_=triu_f, compare_op=mybir.AluOpType.is_gt,
                fill=1.0, base=0, pattern=[[-1, P]], channel_multiplier=1)
            triu = const.tile([P, P], F32R)
            nc.vector.tensor_copy(triu, triu_f)
            ones = const.tile([P, P], F32R)
            onesf = const.tile([P, P], F32)
            nc.vector.memset(onesf, 1.0)
            nc.vector.tensor_copy(ones, onesf)
            zeros8 = const.tile([P, NTB], F32)
            nc.vector.memset(zeros8, 0.0)

            import contextlib
            loop_ctx = (tc.For_i(0, loop_n, 1) if loop_n
                        else contextlib.nullcontext())
            with loop_ctx:
                ys = []
                w_alls = []
                # ---- phase A: logits -> weights w, weighted values y ----
                for pp in range(NP):
                    i0 = 2 * pp
                    if i0 % NTB == 0:
                        w_all = wap.tile([P, NTB], F32)
                        w_alls.append(w_all)

                    ps_v2 = ps_v_pool.tile([P, 2, F], F32)
                    ldeng = nc.sync
                    xT2 = xtp.tile([P, 2, F], F32R)
                    ldeng.dma_start(
                        out=xT2,
                        in_=xt_d.ap()[i0:i0 + 2].rearrange("j p f -> p j f"))
                    xt2 = xp.tile([P, 2, F], F32)
                    ldeng.dma_start(
                        out=xt2,
                        in_=x_d.ap()[i0:i0 + 2].rearrange("j p f -> p j f"))
                    for j in range(2):
                        for k in range(KC):
                            nc.tensor.matmul(
                                ps_v2[:, j, :],
                                xT2[:, j, k * P:(k + 1) * P],
                                W_sb[:, k, :],
                                start=(k == 0),
                                stop=(k == KC - 1 and not have_b),
                            )
                        if have_b:
                            nc.tensor.matmul(ps_v2[:, j, :], ones_row, b_sb,
                                             start=False, stop=True)
                    xts = [xt2[:, 0, :], xt2[:, 1, :]]

                    v2 = vp.tile([P, 2, F], F32)
                    nc.scalar.activation(out=v2, in_=ps_v2, func=Tanh)
                    scr2 = scrp.tile([P, 2, F], F32)
                    nc.gpsimd.tensor_mul(scr2, v2, u_bc2)
                    alpha2 = smal.tile([P, 2], F32)
                    nc.vector.tensor_reduce(alpha2, scr2,
                                            axis=mybir.AxisListType.X, op=ADD)
                    ib0 = i0 % NTB
                    nc.scalar.activation(out=w_all[:, ib0:ib0 + 2],
                                         in_=alpha2, func=Exp)
                    if have_mask:
                        mt = smal.tile([P, 2], F32)
                        for j in range(2):
                            nc.sync.dma_start(out=mt[:, j:j + 1],
                                              in_=m_d.ap()[i0 + j])
                        nc.vector.tensor_mul(w_all[:, ib0:ib0 + 2],
                                             w_all[:, ib0:ib0 + 2], mt)
                    for j in range(2):
                        y = yp.tile([P, F], F32R)
                        nc.vector.tensor_scalar_mul(
                            y, xts[j], w_all[:, ib0 + j:ib0 + j + 1])
                        ys.append(y)

                # ---- phase B: Z prefixes then blockwise cumsum ----
                recs = []
                for batch in range(B_LOC):
                    wr_all = wap.tile([P, NTB], F32R, tag="wr")
                    nc.vector.tensor_copy(wr_all, w_alls[batch])
                    ps_A = ps_Z_pool.tile([P, NTB], F32)
                    ps_B = ps_Z_pool.tile([P, NTB], F32)
                    nc.tensor.matmul(ps_A, triu, wr_all, start=True, stop=True)
                    nc.tensor.matmul(ps_B, ones, wr_all, start=True, stop=True)
                    S = smal.tile([P, NTB], F32)
                    nc.vector.tensor_tensor_scan(
                        out=S, data0=ps_B, data1=zeros8, initial=0.0,
                        op0=ADD, op1=ADD)
                    D = smal.tile([P, NTB], F32)
                    nc.vector.tensor_tensor(out=D, in0=S, in1=ps_B, op=SUB)
                    Z = smal.tile([P, NTB], F32)
                    nc.vector.tensor_tensor(out=Z, in0=D, in1=ps_A, op=ADD)
                    zr = smal.tile([P, NTB], F32)
                    nc.vector.tensor_scalar_add(zr, Z, EPS)
                    rec = smal.tile([P, NTB], F32, tag="rec")
                    nc.vector.reciprocal(rec, zr)
                    recs.append(rec)

                cs = []
                for i in range(NT):
                    ib = i % NTB
                    base = i - ib
                    batch = i // NTB
                    ps_P = ps_P_pool.tile([P, F], F32)
                    nc.tensor.matmul(ps_P, triu, ys[i], start=True,
                                     stop=(ib == 0))
                    for j in range(ib):
                        nc.tensor.matmul(ps_P, ones, ys[base + j],
                                         start=False, stop=(j == ib - 1))
                    if i % 2 == 0:
                        c2 = cp.tile([P, 2, F], F32)
                        cs.append(c2)
                    nc.scalar.activation(out=cs[-1][:, i % 2, :], in_=ps_P,
                                         func=Copy,
                                         scale=recs[batch][:, ib:ib + 1])
                    if i % 2 == 1:
                        nc.scalar.dma_start(
                            out=c_d.ap()[i - 1:i + 1].rearrange(
                                "j p f -> p j f"),
                            in_=cs[-1])

    nc.compile()
    return nc


_NC_CACHE: dict = {}


def _get_nc(have_b, have_mask, loop_n=0):
    key = (have_b, have_mask, loop_n)
    if key not in _NC_CACHE:
        _NC_CACHE[key] = _build(have_b, have_mask, loop_n)
    return _NC_CACHE[key]


def _host_xt(xs):
    """xs: (NT, P, F) tile-major core shard -> pre-transposed layout where
    xt[i, p, k*128+t] = xs[i, t, k*128+p] (chunk-transposed for matmul lhsT)."""
    v = xs.reshape(NT, P, KC, P).transpose(0, 3, 2, 1)
    return np.ascontiguousarray(v).reshape(NT, P, F)


def make_core_maps(x, W, u, b=None, mask_f=None):
    """Build the 8 per-core input maps from full inputs."""
    # W_host[p, k*F + f] = W[k*P + p, f]
    W_r = np.ascontiguousarray(
        W.reshape(KC, P, F).transpose(1, 0, 2).reshape(P, KC * F))
    u_r = np.ascontiguousarray(u.reshape(1, F))
    maps = []
    for core in range(NCORES):
        xs = np.ascontiguousarray(
            x[core * B_LOC:(core + 1) * B_LOC].reshape(NT, P, F))
        m = {"x": xs, "xT": _host_xt(xs), "W": W_r, "u": u_r}
        if b is not None:
            m["b"] = np.ascontiguousarray(b.reshape(1, F))
        if mask_f is not None:
            m["m"] = np.ascontiguousarray(
                mask_f[core * B_LOC:(core + 1) * B_LOC].reshape(NT, P, 1))
        maps.append(m)
    return maps


def kernel(x, mask, W, b, u):
    x = np.asarray(x, dtype=np.float32)
    W = np.asarray(W, dtype=np.float32)
    b = np.asarray(b, dtype=np.float32)
    u = np.asarray(u, dtype=np.float32)
    mask_f = np.asarray(mask).astype(np.float32)

    have_b = bool(np.any(b != 0.0))
    have_mask = bool(np.any(mask_f != 1.0))

    nc = _get_nc(have_b, have_mask)
    in_maps = make_core_maps(x, W, u,
                             b if have_b else None,
                             mask_f if have_mask else None)
    res = run_bass_kernel_spmd(nc, in_maps, core_ids=list(range(NCORES)))
    out = np.stack([r["c"].reshape(B_LOC, T, F) for r in res.results])
    return out.reshape(B, T, F)
